# revision 1
# baseline (speedup 1.0000x reference)
"""DeltaSynapse message-passing kernel for Trainium2 (8 NeuronCores).

Computes I = einsum('eo,dbe,deo,dbe->bo', signs*W, Xd, delaymap, Wshort+1)
with the post dimension (o) sharded across 8 cores.

Math note: reference signs = where(W>0, 2*signs_pre-1, 0) and W >= 0, so
signs*W == (2*signs_pre-1)*W exactly (where W==0 both sides are 0). We fold
the sign vector s[e] into the small tensor A'[d,b,e] = Xd*(Wshort+1)*s[e],
so the big W matrix is used raw:
    I[b,o] = sum_{d,e} A'[d,b,e] * (W[e,o] * delaymap[d,e,o])

Per-core plan (o-shard of 256 columns):
  - delaymap shard (8 x 2048 x 256) is cast to bf16 on the host (it is
    binary {0,1} -> exact in bf16) and streamed as 8 x 1MB chunk DMAs
    (one per pair of 128-row e-chunks, host-relayouted so every DMA is
    contiguous with 8KB per partition).
  - W shard (2048 x 256, bf16) stays resident in SBUF and is replicated
    over the delay dim once on-chip (w_rep); the DVE multiplies each
    incoming chunk by w_rep with one large plain contiguous bf16
    tensor_tensor per chunk group (broadcast/in-place DVE forms measured
    pathologically slow on real HW).
  - The PE accumulates 128 bf16 matmuls (K=128 e's, M=16 batch, N=256
    posts) into a single PSUM tile (fp32 accumulate).
  - A' = Xd*(Wshort+1)*s is computed on-device in fp32 from the raw
    replicated inputs, then rounded once to bf16 for the matmul lhsT.
"""

import numpy as np

import concourse.bass as bass  # noqa: F401
import concourse.mybir as mybir
from concourse import bacc
from concourse.bass_utils import run_bass_kernel_spmd
from concourse.tile import TileContext

D, B, E, O = 8, 16, 2048, 2048
NCORES = 8
P = 128
O_SH = O // NCORES  # 256 post columns per core
EC = E // P  # 16 e-chunks
G = EC // 2  # chunk pairs (1MB bf16 DMAs)

_NC_CACHE = {}


def _build(loop_iters=None):
    f32 = mybir.dt.float32
    bf16 = mybir.dt.bfloat16

    nc = bacc.Bacc("TRN2", target_bir_lowering=False, debug=False)
    AUX = 2 * EC * D * B + EC  # xd | ws | s rows per partition
    x_dm = nc.dram_tensor("dm", [EC, P, D * O_SH], bf16, kind="ExternalInput")
    x_w = nc.dram_tensor("w", [P, EC * O_SH], bf16, kind="ExternalInput")
    x_aux = nc.dram_tensor("aux", [P, AUX], bf16, kind="ExternalInput")
    y = nc.dram_tensor("y", [B, O_SH], f32, kind="ExternalOutput")

    with TileContext(nc) as tc:
        with (
            tc.tile_pool(name="const", bufs=1) as const,
            tc.tile_pool(name="dmp", bufs=6) as dmp,
            tc.tile_pool(name="mp", bufs=3) as mp,
            tc.tile_pool(name="psp", bufs=1, space="PSUM") as psp,
            tc.tile_pool(name="outp", bufs=1) as outp,
        ):

            def body(_i=None):
                # aux (gates A' -> every matmul) and W (gates every chunk
                # multiply) go first, ahead of the delaymap stream.
                aux_t = const.tile([P, AUX], bf16)
                nc.scalar.dma_start(out=aux_t[:], in_=x_aux.ap())
                w_t = const.tile([P, EC, O_SH], bf16)
                nc.sync.dma_start(out=w_t[:], in_=x_w.ap())
                xd_v = aux_t[:, : EC * D * B].rearrange(
                    "p (c x) -> p c x", c=EC
                )
                ws_v = aux_t[:, EC * D * B : 2 * EC * D * B].rearrange(
                    "p (c x) -> p c x", c=EC
                )
                s_v = aux_t[:, 2 * EC * D * B :]  # (P, EC)

                # A' = (Wshort + 1) * Xd * s[e]   (e on partitions+chunks)
                a_t = const.tile([P, EC, D * B], bf16)
                nc.vector.tensor_scalar_add(a_t[:], ws_v, 1.0)
                nc.vector.tensor_tensor(
                    a_t[:], a_t[:], xd_v, mybir.AluOpType.mult
                )
                a_r = const.tile([P, EC, D * B], bf16)
                nc.vector.tensor_tensor(
                    a_r[:],
                    a_t[:],
                    s_v[:, :, None].to_broadcast((P, EC, D * B)),
                    mybir.AluOpType.mult,
                )

                # Replicate W over the delay dim once on the (otherwise
                # idle) ACT engine so every chunk multiply is one large
                # PLAIN contiguous tensor_tensor — broadcast/in-place DVE
                # forms measured pathologically slow on HW.
                w_rep = const.tile([P, EC, D, O_SH], bf16)
                for d in range(D):
                    nc.scalar.copy(out=w_rep[:, :, d, :], in_=w_t[:])

                ps = psp.tile([B, O_SH], f32)
                # Chunk DMAs: a single first chunk (earlier pipeline start),
                # 1MB pairs in the middle, and a half-chunk tail (shorter
                # post-last-byte chain): (chunk, n_chunks, d0, nd).
                groups = [(0, 1, 0, D), (1, 2, 0, D), (3, 2, 0, D)]
                groups += [(c0, 2, 0, D) for c0 in range(5, EC - 1, 2)]
                groups += [(EC - 1, 1, 0, D // 2), (EC - 1, 1, D // 2, D // 2)]
                for gi, (c0, n, d0, nd) in enumerate(groups):
                    dm_t = dmp.tile([P, n, nd, O_SH], bf16, tag="dm")
                    src = x_dm.ap().rearrange(
                        "c p (d o) -> c p d o", o=O_SH
                    )[c0 : c0 + n, :, d0 : d0 + nd]
                    eng = nc.scalar if gi % 2 else nc.sync
                    eng.dma_start(
                        out=dm_t[:], in_=src.rearrange("c p d o -> p c d o")
                    )
                    # M = delaymap * W_rep: one large plain contiguous TT
                    m_t = mp.tile([P, n, nd, O_SH], bf16, tag="m")
                    nc.vector.tensor_tensor(
                        m_t[:],
                        dm_t[:],
                        w_rep[:, c0 : c0 + n, d0 : d0 + nd, :],
                        mybir.AluOpType.mult,
                    )
                    for cc in range(n):
                        c = c0 + cc
                        for d in range(d0, d0 + nd):
                            nc.tensor.matmul(
                                ps[:],
                                a_r[:, c, d * B : (d + 1) * B],
                                m_t[:, cc, d - d0, :],
                                start=(c == 0 and d == 0),
                                stop=(c == EC - 1 and d == D - 1),
                            )

                o_t = outp.tile([B, O_SH], f32)
                nc.vector.tensor_copy(out=o_t[:], in_=ps[:])
                nc.scalar.dma_start(out=y.ap(), in_=o_t[:])

            if loop_iters is None:
                body()
            else:
                # PE loop body is ~258 instructions (>1 IRAM block): arm the
                # back-edge branch prefetch so each iteration doesn't pay an
                # I$-miss DMA fetch.
                with tc.For_i(
                    0, loop_iters, 1, hint_engines=(mybir.EngineType.PE,)
                ) as i:
                    body(i)

    nc.compile()
    return nc


def _get_nc(loop_iters=None):
    key = loop_iters
    if key not in _NC_CACHE:
        _NC_CACHE[key] = _build(loop_iters)
    return _NC_CACHE[key]


def _make_in_maps(W, Xd, delaymap, Wshort, signs_pre):
    import ml_dtypes

    bf16 = ml_dtypes.bfloat16
    W = np.asarray(W, dtype=np.float32)
    Xd = np.asarray(Xd, dtype=np.float32)
    delaymap = np.asarray(delaymap, dtype=np.float32)
    Wshort = np.asarray(Wshort, dtype=np.float32)
    signs_pre = np.asarray(signs_pre)

    s = (2 * signs_pre - 1).astype(np.float32)  # (E,)
    s_re = s.reshape(EC, P).T  # [p, c]
    xd_re = (
        Xd.reshape(D, B, EC, P).transpose(3, 2, 0, 1).reshape(P, EC * D * B)
    )
    ws_re = (
        Wshort.reshape(D, B, EC, P).transpose(3, 2, 0, 1).reshape(P, EC * D * B)
    )
    # merged A'-inputs: xd | ws | s, one contiguous bf16 row per partition
    aux_re = np.ascontiguousarray(
        np.concatenate([xd_re, ws_re, s_re], axis=1).astype(bf16)
    )

    in_maps = []
    for i in range(NCORES):
        o0 = i * O_SH
        w_re = np.ascontiguousarray(
            W[:, o0 : o0 + O_SH]
            .reshape(EC, P, O_SH)
            .transpose(1, 0, 2)
            .reshape(P, EC * O_SH)
            .astype(bf16)
        )
        # [c, p, (d, o)]: per e-chunk, per-partition-contiguous rows
        dm_re = np.ascontiguousarray(
            delaymap[:, :, o0 : o0 + O_SH]
            .reshape(D, EC, P, O_SH)
            .transpose(1, 2, 0, 3)  # (EC, P, D, O_SH)
            .reshape(EC, P, D * O_SH)
            .astype(bf16)
        )
        in_maps.append({"dm": dm_re, "w": w_re, "aux": aux_re})
    return in_maps


def run(W, Xd, delaymap, Wshort, signs_pre, loop_iters=None):
    """Run on the 8 NeuronCores; returns (I, BassKernelResults)."""
    nc = _get_nc(loop_iters)
    in_maps = _make_in_maps(W, Xd, delaymap, Wshort, signs_pre)
    res = run_bass_kernel_spmd(nc, in_maps, core_ids=list(range(NCORES)))
    I = np.concatenate(
        [res.results[i]["y"] for i in range(NCORES)], axis=1
    ).astype(np.float32)
    return I, res


def kernel(W, Xd, delaymap, Wshort, signs_pre):
    I, _ = run(W, Xd, delaymap, Wshort, signs_pre)
    return I



# revision 2
# speedup vs baseline: 1.1576x; 1.1576x over previous
"""DeltaSynapse message-passing kernel for Trainium2 (8 NeuronCores).

Computes I = einsum('eo,dbe,deo,dbe->bo', signs*W, Xd, delaymap, Wshort+1)
with the post dimension (o) sharded across 8 cores.

Math: reference signs = where(W>0, 2*signs_pre-1, 0) and W >= 0, so
signs*W == s*W with s = 2*signs_pre-1 (where W==0 both sides are 0). The
sign s[e] is folded into the host-side layout of Xd (Xd_signed = Xd*s),
so on device:
    I[b,o] = sum_{d,e} [(Wshort+1)*Xd_signed][d,b,e] * (delaymap*W)[d,e,o]

Per-core plan (o-shard of 256 columns), d-major streaming:
  - delaymap shard is binary -> stored fp8e4 in HBM (exact) and streamed
    by gpsimd SWDGE cast-DMAs (fp8 -> bf16) in 4 pair-of-delay chunks
    (1 MB HBM each). All cast-DMA descriptor generation is issued ahead
    of any Pool compute so the ring never stalls on a semaphore.
  - Queue placement (measured): the SWDGE ring sustains ~570 GB/s
    SBUF-side and carries w + the delaymap stream; the slow HWDGE
    queues carry only aux (scalar) and the 16 KB y writeback (sync).
    The y writeback's matmul-wait must not gate any other transfer, so
    sync carries nothing else.
  - m[d] = delaymap[d] * W as flat [P, 4096] bf16 tensor_tensors on the
    DVE: flat single-dim APs engage the DVE 2x packed mode (~224 G
    elem/s measured; sliced 3-dim views run at 1x).
  - A' = (Wshort+1)*Xd_signed is one fused DVE scalar_tensor_tensor.
  - PE: 128 bf16 matmuls (K=128 e's, M=16 batch, N=256 posts) on two
    interleaved PSUM accumulation chains (even/odd e-chunk), hiding the
    ~70 ns per-matmul latency bubble (154 vs 208 ns/matmul measured);
    DVE combines the two chains and sync DMAs the result out.
"""

import numpy as np

import concourse.bass as bass  # noqa: F401
import concourse.mybir as mybir
from concourse import bacc
from concourse.bass_utils import run_bass_kernel_spmd
from concourse.tile import TileContext

D, B, E, O = 8, 16, 2048, 2048
NCORES = 8
P = 128
O_SH = O // NCORES  # 256 post columns per core
EC = E // P  # 16 e-chunks
PAIRS = D // 2  # delaymap DMA granularity: 2 delays per transfer

_NC_CACHE = {}


def _build(loop_iters=None):
    f32 = mybir.dt.float32
    bf16 = mybir.dt.bfloat16
    fp8 = mybir.dt.float8e4

    nc = bacc.Bacc("TRN2", target_bir_lowering=False, debug=False)
    x_dm = nc.dram_tensor(
        "dm", [PAIRS, P, 2 * EC * O_SH], fp8, kind="ExternalInput"
    )
    x_w = nc.dram_tensor("w", [P, EC * O_SH], bf16, kind="ExternalInput")
    x_aux = nc.dram_tensor(
        "aux", [P, 2, EC, D * B], bf16, kind="ExternalInput"
    )
    y = nc.dram_tensor("y", [B, O_SH], f32, kind="ExternalOutput")

    with TileContext(nc) as tc:
        with (
            tc.tile_pool(name="const", bufs=3) as const,
            tc.tile_pool(name="dmp", bufs=5) as dmp,
            tc.tile_pool(name="mp", bufs=4) as mp,
            tc.tile_pool(name="psp", bufs=2, space="PSUM") as psp,
            tc.tile_pool(name="outp", bufs=2) as outp,
        ):

            def body(_i=None):
                aux_t = const.tile([P, 2, EC, D * B], bf16)
                w_t = const.tile([P, EC, O_SH], bf16)
                # ring: w first (every multiply consumes it), then the
                # delaymap stream; scalar: aux; sync: y only
                nc.gpsimd.dma_start(out=w_t[:], in_=x_w.ap())
                nc.scalar.dma_start(out=aux_t[:], in_=x_aux.ap())

                dm_ts = []
                for g in range(PAIRS):
                    dm_t = dmp.tile([P, 2, EC, O_SH], bf16, tag="dm")
                    nc.gpsimd.dma_start(out=dm_t[:], in_=x_dm.ap()[g])
                    dm_ts.append(dm_t)

                # A' = (Wshort + 1) * (Xd * s)  (sign pre-folded on host)
                a_r = const.tile([P, EC, D, B], bf16)
                nc.vector.scalar_tensor_tensor(
                    a_r[:].rearrange("p c d b -> p (c d b)"),
                    aux_t[:, 1].rearrange("p c x -> p (c x)"),
                    1.0,
                    aux_t[:, 0].rearrange("p c x -> p (c x)"),
                    mybir.AluOpType.add,
                    mybir.AluOpType.mult,
                )

                ps0 = psp.tile([B, O_SH], f32, name="ps0", tag="ps0")
                ps1 = psp.tile([B, O_SH], f32, name="ps1", tag="ps1")
                for g in range(PAIRS):
                    dm_t = dm_ts[g]
                    for j in range(2):
                        d = 2 * g + j
                        m_t = mp.tile([P, EC, O_SH], bf16, tag="m")
                        # flat APs: DVE 2x packed mode needs single-dim
                        # free access patterns
                        nc.vector.tensor_tensor(
                            m_t[:].rearrange("p c o -> p (c o)"),
                            dm_t[:, j].rearrange("p c o -> p (c o)"),
                            w_t[:].rearrange("p c o -> p (c o)"),
                            mybir.AluOpType.mult,
                        )
                        for c in range(EC):
                            nc.tensor.matmul(
                                (ps0 if c % 2 == 0 else ps1)[:],
                                a_r[:, c, d, :],
                                m_t[:, c, :],
                                start=(d == 0 and c < 2),
                                stop=(d == D - 1 and c >= EC - 2),
                            )

                o_t = outp.tile([B, O_SH], f32)
                # chain-combine on DVE (idle at iteration end; an ACT copy
                # here would gate the next aux issue behind the matmuls)
                nc.vector.tensor_copy(out=o_t[:], in_=ps0[:])
                nc.vector.tensor_tensor(
                    o_t[:], o_t[:], ps1[:], mybir.AluOpType.add
                )
                nc.sync.dma_start(out=y.ap(), in_=o_t[:])

            if loop_iters is None:
                body()
            else:
                with tc.For_i(
                    0, loop_iters, 1, hint_engines=(mybir.EngineType.PE,)
                ) as i:
                    body(i)

    nc.compile()
    return nc


def _get_nc(loop_iters=None):
    if loop_iters not in _NC_CACHE:
        _NC_CACHE[loop_iters] = _build(loop_iters)
    return _NC_CACHE[loop_iters]


def _make_in_maps(W, Xd, delaymap, Wshort, signs_pre):
    import ml_dtypes

    bf16 = ml_dtypes.bfloat16
    fp8 = ml_dtypes.float8_e4m3fn
    W = np.asarray(W, dtype=np.float32)
    Xd = np.asarray(Xd, dtype=np.float32)
    Wshort = np.asarray(Wshort, dtype=np.float32)
    signs_pre = np.asarray(signs_pre)

    s = (2 * signs_pre - 1).astype(np.float32)  # (E,)
    s_re = s.reshape(EC, P).T  # (P, EC)
    # (P, EC, D, B) views of Xd/Wshort; sign folded into Xd
    xd_re = Xd.reshape(D, B, EC, P).transpose(3, 2, 0, 1)
    xd_re = xd_re * s_re[:, :, None, None]
    ws_re = Wshort.reshape(D, B, EC, P).transpose(3, 2, 0, 1)
    aux_re = np.ascontiguousarray(
        np.stack([xd_re, ws_re], axis=1).astype(bf16)
    ).reshape(P, 2, EC, D * B)

    # binary delaymap -> fp8e4m3 via byte trick (0.0 -> 0x00, 1.0 -> 0x38):
    # exact and ~10x faster than a float cast on the host
    dm8 = (np.asarray(delaymap) != 0).astype(np.uint8) * np.uint8(0x38)

    in_maps = []
    for i in range(NCORES):
        o0 = i * O_SH
        w_re = np.ascontiguousarray(
            W[:, o0 : o0 + O_SH]
            .reshape(EC, P, O_SH)
            .transpose(1, 0, 2)
            .reshape(P, EC * O_SH)
            .astype(bf16)
        )
        # (PAIRS, P, 2*EC*O_SH): per pair-of-delays, per-partition rows
        # hold both delays' (EC, O_SH) blocks contiguously
        dm_re = (
            np.ascontiguousarray(
                dm8[:, :, o0 : o0 + O_SH]
                .reshape(PAIRS, 2, EC, P, O_SH)
                .transpose(0, 3, 1, 2, 4)
                .reshape(PAIRS, P, 2 * EC * O_SH)
            )
            .view(fp8)
        )
        in_maps.append({"dm": dm_re, "w": w_re, "aux": aux_re})
    return in_maps


def run(W, Xd, delaymap, Wshort, signs_pre, loop_iters=None):
    """Run on the 8 NeuronCores; returns (I, BassKernelResults)."""
    nc = _get_nc(loop_iters)
    in_maps = _make_in_maps(W, Xd, delaymap, Wshort, signs_pre)
    res = run_bass_kernel_spmd(nc, in_maps, core_ids=list(range(NCORES)))
    I = np.concatenate(
        [res.results[i]["y"] for i in range(NCORES)], axis=1
    ).astype(np.float32)
    return I, res


def kernel(W, Xd, delaymap, Wshort, signs_pre):
    I, _ = run(W, Xd, delaymap, Wshort, signs_pre)
    return I
